# revision 1
# baseline (speedup 1.0000x reference)
"""nn_CausalSelfAttention_88854283420050 — Bass/Tile kernel for 8 trn2 cores.

Sharding: tensor-parallel over heads (H=16 -> 2 heads per core).
Each core computes, for its 2 heads: the qkv projection (columns of
c_attn), per-head LayerNorm + RoPE, causal attention, and a partial
output projection y_c = O_heads @ W_proj[:, head cols].T.  The host
sums the 8 partial projections (row-parallel c_proj) and adds b_proj.

Device program (identical SPMD program on all 8 cores, per-core weights):
  Phase A: qkv = x @ Wqkv_c.T (+bias via an appended ones-row of x),
           natural [t, 768] layout; LN stats + apply + RoPE on DVE/ACT;
           PE-transpose of q,k into [c, t] layout; v kept natural [s, c].
  Phase B: per head, per 512-wide t-block: S^T tiles = k_tile^T-stat x q
           (fp32r matmuls), exp on ACT (scores are bounded by sqrt(C)
           after LN so no max-subtraction is needed), causal masking via
           precomputed 0/1 masks on the 4 diagonal tiles, row-sums L via
           a ones-column matmul, O^T accumulation with v stationary,
           normalization by 1/L broadcast through a rank-1 matmul.
  Phase C: y[t, d] partial = sum_h O_h^T-stat x W_proj-cols, DMA out.

All matmuls run in float32r (fp22 mantissa truncation on read, full
fp32 accumulation in PSUM) — full PE rate with ~6e-5 relative error.
"""
import math
import os
import sys

sys.path.insert(0, "/opt/trn_rl_repo")

import numpy as np
from concourse import bacc, mybir, tile
from concourse import bass_utils

T, D, H, C = 2048, 2048, 16, 128
EPS = 1e-6
NCORES = 8
HPC = H // NCORES  # heads per core
DT = 17            # contraction tiles incl. bias row
F32 = mybir.dt.float32
F32R = mybir.dt.float32r
AF = mybir.ActivationFunctionType
ALU = mybir.AluOpType
AX = mybir.AxisListType

NT = T // 128
NB = T // 512

_NC_CACHE = None
LAST_RESULT = None


def _build_program():
    nc = bacc.Bacc("TRN2", target_bir_lowering=False, debug=False,
                   enable_asserts=True, num_devices=NCORES)

    xT = nc.dram_tensor("xT", [DT * 128, T], F32R, kind="ExternalInput").ap()
    wqkv = nc.dram_tensor("wqkv", [DT * 128, 6 * C], F32R, kind="ExternalInput").ap()
    ropecos = nc.dram_tensor("ropecos", [T, 4 * C], F32, kind="ExternalInput").ap()
    ropesin = nc.dram_tensor("ropesin", [T, 4 * C], F32, kind="ExternalInput").ap()
    masks = nc.dram_tensor("masks", [128, 4 * 512], F32, kind="ExternalInput").ap()
    wp = nc.dram_tensor("wp", [HPC * C, D], F32R, kind="ExternalInput").ap()
    onescol = nc.dram_tensor("onescol", [128, 1], F32R, kind="ExternalInput").ap()
    onesrow = nc.dram_tensor("onesrow", [1, 128], F32R, kind="ExternalInput").ap()
    ident = nc.dram_tensor("ident", [128, 128], F32, kind="ExternalInput").ap()
    y = nc.dram_tensor("y", [T, D], F32, kind="ExternalOutput").ap()

    sc = 1.0 / math.sqrt(C)

    with tile.TileContext(nc) as tc:
        with tc.tile_pool(name="res", bufs=1) as res:
            qT = res.tile([128, HPC, T], F32R, tag="qT")       # [c, h, t]
            kT = res.tile([128, HPC, T], F32R, tag="kT")
            vv = res.tile([128, HPC, NT, C], F32R, tag="vv")   # [s, h, stile, c]
            ot = res.tile([128, HPC, T], F32R, tag="ot")       # [c, h, t]
            ones_c = res.tile([128, 1], F32R, tag="onescol")
            ones_r = res.tile([1, 128], F32R, tag="onesrow")
            id_sb = res.tile([128, 128], F32, tag="ident")

            zeros_c = res.tile([128, 1], F32, tag="zeros_c")
            eps_c = res.tile([128, 1], F32, tag="eps_c")
            nc.gpsimd.memset(zeros_c[:], 0.0)
            nc.gpsimd.memset(eps_c[:], EPS)
            nc.sync.dma_start(ones_c[:], onescol[:])
            nc.sync.dma_start(ones_r[:], onesrow[:])
            nc.sync.dma_start(id_sb[:], ident[:])

            # =========== Phase A: QKV projection + LN + RoPE ===========
            with (
                tc.tile_pool(name="wq", bufs=1) as wqp,
                tc.tile_pool(name="qn", bufs=1) as qnp,
            ):
                qn_all = qnp.tile([128, NT, 4 * C], F32, tag="qn_all")

                w_sb = wqp.tile([128, DT, 6 * C], F32R, tag="w_sb")
                nc.sync.dma_start(
                    w_sb[:], wqkv.rearrange("(a p) n -> p a n", p=128))

                with (
                    tc.tile_pool(name="xcol", bufs=2) as xcolp,
                    tc.tile_pool(name="psA", bufs=2, space="PSUM") as psAp,
                    tc.tile_pool(name="psB", bufs=2, space="PSUM") as psBp,
                    tc.tile_pool(name="rope", bufs=2) as ropep,
                    tc.tile_pool(name="lnst", bufs=2) as lnstp,
                    tc.tile_pool(name="sq", bufs=2) as sqp,
                    tc.tile_pool(name="rot", bufs=2) as rotp,
                ):
                    for tt in range(NT):
                        xcol = xcolp.tile([128, DT, 128], F32R, tag="xcol")
                        nc.sync.dma_start(
                            xcol[:],
                            xT.rearrange("(a p) t -> p a t", p=128)[
                                :, :, tt * 128:(tt + 1) * 128])
                        psA = psAp.tile([128, 512], F32, tag="psA")
                        psB = psBp.tile([128, 256], F32, tag="psB")
                        for dt in range(DT):
                            nc.tensor.matmul(
                                psA[:], xcol[:, dt, :], w_sb[:, dt, 0:512],
                                start=(dt == 0), stop=(dt == DT - 1))
                            nc.tensor.matmul(
                                psB[:], xcol[:, dt, :], w_sb[:, dt, 512:768],
                                start=(dt == 0), stop=(dt == DT - 1))
                        for h in range(HPC):
                            nc.scalar.activation(
                                vv[:, h, tt, :], psB[:, h * C:(h + 1) * C],
                                AF.Copy)
                        rc = ropep.tile([128, 512], F32, tag="ropec")
                        rs = ropep.tile([128, 512], F32, tag="ropes")
                        nc.sync.dma_start(
                            rc[:], ropecos[tt * 128:(tt + 1) * 128, :])
                        nc.sync.dma_start(
                            rs[:], ropesin[tt * 128:(tt + 1) * 128, :])
                        st = lnstp.tile([128, 16], F32, tag="lnst")
                        # st cols: 0:4 sums, 4:8 sumsq, 8:12 rstd, 12:16 -mu*rstd
                        nc.vector.reduce_sum(
                            st[:, 0:4],
                            psA[:].rearrange("p (a b) -> p a b", a=4),
                            axis=AX.X)
                        for i in range(4):
                            sq = sqp.tile([128, 128], F32, tag="sq")
                            nc.scalar.activation(
                                sq[:], psA[:, i * 128:(i + 1) * 128], AF.Square,
                                bias=zeros_c[:], accum_out=st[:, 4 + i:5 + i])
                        mu = lnstp.tile([128, 8], F32, tag="mu")
                        nc.vector.tensor_scalar(
                            mu[:, 0:8], st[:, 0:8], 1.0 / C, None, op0=ALU.mult)
                        var = lnstp.tile([128, 4], F32, tag="var")
                        nc.vector.tensor_tensor(
                            var[:], mu[:, 0:4], mu[:, 0:4], op=ALU.mult)
                        nc.vector.tensor_tensor(
                            var[:], mu[:, 4:8], var[:], op=ALU.subtract)
                        nc.scalar.activation(var[:], var[:], AF.Sqrt,
                                             bias=eps_c[:])
                        nc.vector.reciprocal(st[:, 8:12], var[:])
                        nc.vector.tensor_tensor(
                            st[:, 12:16], mu[:, 0:4], st[:, 8:12], op=ALU.mult)
                        nc.vector.tensor_scalar(
                            st[:, 12:16], st[:, 12:16], -1.0, None,
                            op0=ALU.mult)
                        qn = qn_all[:, tt, :]
                        for i in range(4):
                            nc.vector.tensor_scalar(
                                qn[:, i * 128:(i + 1) * 128],
                                psA[:, i * 128:(i + 1) * 128],
                                st[:, 8 + i:9 + i], st[:, 12 + i:13 + i],
                                op0=ALU.mult, op1=ALU.add)
                        rot = rotp.tile([128, 512], F32, tag="rot")
                        qn3 = qn.rearrange("p (a b) -> p a b", b=2)
                        rot3 = rot[:].rearrange("p (a b) -> p a b", b=2)
                        nc.vector.tensor_scalar(
                            rot3[:, :, 0], qn3[:, :, 1], -1.0, None,
                            op0=ALU.mult)
                        nc.vector.tensor_copy(rot3[:, :, 1], qn3[:, :, 0])
                        nc.vector.tensor_tensor(qn, qn, rc[:], op=ALU.mult)
                        nc.vector.tensor_tensor(
                            rot[:], rot[:], rs[:], op=ALU.mult)
                        nc.vector.tensor_tensor(qn, qn, rot[:], op=ALU.add)

                # ---- Phase A2: transpose q,k into qT/kT ----
                with tc.tile_pool(name="psT", bufs=4, space="PSUM") as psTp:
                    for tt in range(NT):
                        for i in range(4):
                            psT = psTp.tile([128, 128], F32, tag="psT")
                            nc.tensor.transpose(
                                psT[:], qn_all[:, tt, i * 128:(i + 1) * 128],
                                id_sb[:])
                            dst = qT if i < 2 else kT
                            nc.scalar.activation(
                                dst[:, i % 2, tt * 128:(tt + 1) * 128],
                                psT[:], AF.Copy)

            # =========== Phase B: attention per head/t-block ===========
            with tc.tile_pool(name="resB", bufs=1) as resB:
                masks_sb = resB.tile([128, 4 * 512], F32, tag="masks")
                wp_sb = resB.tile([128, HPC, D], F32R, tag="wp")
                nc.sync.dma_start(masks_sb[:], masks[:])
                nc.sync.dma_start(
                    wp_sb[:], wp.rearrange("(h p) d -> p h d", p=128))

                with (
                    tc.tile_pool(name="psS", bufs=3, space="PSUM") as psSp,
                    tc.tile_pool(name="psL", bufs=2, space="PSUM") as psLp,
                    tc.tile_pool(name="psO", bufs=2, space="PSUM") as psOp,
                    tc.tile_pool(name="psBC", bufs=1, space="PSUM") as psBCp,
                    tc.tile_pool(name="aT", bufs=3) as aTp,
                    tc.tile_pool(name="bsm", bufs=2) as bsmp,
                ):
                    for h in range(HPC):
                        for tb in range(NB):
                            S = 4 * (tb + 1)
                            qTs = qT[:, h, tb * 512:(tb + 1) * 512]
                            st_ps = [None] * S

                            def emit_st(s):
                                stp = psSp.tile([128, 512], F32, tag="psS")
                                nc.tensor.matmul(
                                    stp[:], kT[:, h, s * 128:(s + 1) * 128],
                                    qTs, start=True, stop=True)
                                st_ps[s] = stp

                            Lps = psLp.tile([1, 512], F32, tag="psL")
                            Ops = psOp.tile([128, 512], F32, tag="psO")
                            emit_st(0)
                            for s in range(S):
                                if s + 1 < S:
                                    emit_st(s + 1)
                                a = aTp.tile([128, 512], F32R, tag="aT")
                                nc.scalar.activation(
                                    a[:], st_ps[s][:], AF.Exp,
                                    bias=zeros_c[:], scale=sc)
                                st_ps[s] = None
                                if s >= 4 * tb:
                                    j = s - 4 * tb
                                    nc.vector.tensor_tensor(
                                        a[:], a[:],
                                        masks_sb[:, j * 512:(j + 1) * 512],
                                        op=ALU.mult)
                                nc.tensor.matmul(
                                    Lps[:], ones_c[:], a[:],
                                    start=(s == 0), stop=(s == S - 1))
                                nc.tensor.matmul(
                                    Ops[:], vv[:, h, s, :], a[:],
                                    start=(s == 0), stop=(s == S - 1))
                            recL = bsmp.tile([1, 512], F32, tag="recL")
                            nc.vector.reciprocal(recL[:], Lps[:])
                            recLr = bsmp.tile([1, 512], F32R, tag="recLr")
                            nc.scalar.activation(recLr[:], recL[:], AF.Copy)
                            bc = psBCp.tile([128, 512], F32, tag="psBC")
                            nc.tensor.matmul(bc[:], ones_r[:], recLr[:],
                                             start=True, stop=True)
                            bcs = bsmp.tile([128, 512], F32, tag="bcs")
                            nc.scalar.activation(bcs[:], bc[:], AF.Copy)
                            nc.vector.tensor_tensor(
                                ot[:, h, tb * 512:(tb + 1) * 512], Ops[:],
                                bcs[:], op=ALU.mult)

                # =========== Phase C: output projection ===========
                with (
                    tc.tile_pool(name="psY", bufs=2, space="PSUM") as psYp,
                    tc.tile_pool(name="ysb", bufs=3) as ysbp,
                ):
                    for ttt in range(NT):
                        for db in range(NB):
                            yps = psYp.tile([128, 512], F32, tag="psY")
                            for h in range(HPC):
                                nc.tensor.matmul(
                                    yps[:],
                                    ot[:, h, ttt * 128:(ttt + 1) * 128],
                                    wp_sb[:, h, db * 512:(db + 1) * 512],
                                    start=(h == 0), stop=(h == HPC - 1))
                            ysb = ysbp.tile([128, 512], F32, tag="ysb")
                            nc.scalar.activation(ysb[:], yps[:], AF.Copy)
                            nc.sync.dma_start(
                                y[ttt * 128:(ttt + 1) * 128,
                                  db * 512:(db + 1) * 512],
                                ysb[:])

    nc.compile()
    return nc


def _host_prep(x, W_attn, b_attn, W_proj, q_ln_w, k_ln_w):
    f = np.float32
    xT = np.zeros((DT * 128, T), f)
    xT[:D] = x.T
    xT[D] = 1.0

    inv = (1.0 / (10000.0 ** (np.arange(0, C, 2, dtype=f) / C))).astype(f)
    freqs = np.arange(T, dtype=f)[:, None] * inv[None, :]
    sin = np.repeat(np.sin(freqs), 2, axis=1).astype(f)
    cos = np.repeat(np.cos(freqs), 2, axis=1).astype(f)
    part = np.arange(C) ^ 1
    cos_q = cos * q_ln_w[None, :]
    sin_q = sin * q_ln_w[None, part]
    cos_k = cos * k_ln_w[None, :]
    sin_k = sin * k_ln_w[None, part]
    ropecos = np.ascontiguousarray(
        np.concatenate([cos_q, cos_q, cos_k, cos_k], axis=1).astype(f))
    ropesin = np.ascontiguousarray(
        np.concatenate([sin_q, sin_q, sin_k, sin_k], axis=1).astype(f))

    ss = np.arange(128)[:, None]
    ttm = np.arange(512)[None, :]
    masks = np.ascontiguousarray(np.concatenate(
        [(j * 128 + ss <= ttm).astype(f) for j in range(4)], axis=1))

    shared = dict(xT=xT, ropecos=ropecos, ropesin=ropesin, masks=masks,
                  onescol=np.ones((128, 1), f),
                  onesrow=np.ones((1, 128), f),
                  ident=np.eye(128, dtype=f))

    in_maps = []
    for c in range(NCORES):
        h0, h1 = HPC * c, HPC * c + 1
        rows = np.concatenate([
            np.arange(h0 * C, (h0 + 1) * C),
            np.arange(h1 * C, (h1 + 1) * C),
            D + np.arange(h0 * C, (h0 + 1) * C),
            D + np.arange(h1 * C, (h1 + 1) * C),
            2 * D + np.arange(h0 * C, (h0 + 1) * C),
            2 * D + np.arange(h1 * C, (h1 + 1) * C),
        ])
        wqkv = np.zeros((DT * 128, 6 * C), f)
        wqkv[:D] = W_attn[rows].T
        wqkv[D] = b_attn[rows]
        wpc = np.concatenate(
            [W_proj[:, h0 * C:(h0 + 1) * C].T,
             W_proj[:, h1 * C:(h1 + 1) * C].T], axis=0)
        m = dict(shared)
        m["wqkv"] = np.ascontiguousarray(wqkv)
        m["wp"] = np.ascontiguousarray(wpc)
        in_maps.append(m)
    return in_maps


def kernel(x, W_attn, b_attn, W_proj, b_proj, q_ln_w, k_ln_w):
    global _NC_CACHE, LAST_RESULT
    f = np.float32
    x = np.ascontiguousarray(np.asarray(x, f))
    W_attn = np.ascontiguousarray(np.asarray(W_attn, f))
    b_attn = np.ascontiguousarray(np.asarray(b_attn, f))
    W_proj = np.ascontiguousarray(np.asarray(W_proj, f))
    b_proj = np.ascontiguousarray(np.asarray(b_proj, f))
    q_ln_w = np.ascontiguousarray(np.asarray(q_ln_w, f))
    k_ln_w = np.ascontiguousarray(np.asarray(k_ln_w, f))

    in_maps = _host_prep(x, W_attn, b_attn, W_proj, q_ln_w, k_ln_w)
    if _NC_CACHE is None:
        _NC_CACHE = _build_program()
    nc = _NC_CACHE

    res = bass_utils.run_bass_kernel_spmd(
        nc, in_maps, core_ids=list(range(NCORES)),
        trace=bool(os.environ.get("BASS_TRACE")))
    LAST_RESULT = res

    y = np.zeros((T, D), np.float32)
    for rmap in res.results:
        y += rmap["y"]
    y += b_proj[None, :]
    return y



# revision 4
# speedup vs baseline: 1.4066x; 1.4066x over previous
"""nn_CausalSelfAttention_88854283420050 — Bass/Tile kernel for 8 trn2 cores.

Sharding: tensor-parallel over heads (H=16 -> 2 heads per core).
Each core computes, for its 2 heads: the qkv projection (columns of
c_attn), per-head LayerNorm + RoPE, causal attention, and a partial
output projection y_c = O_heads @ W_proj[:, head cols].T.  The host
sums the 8 partial projections (row-parallel c_proj) and adds b_proj.

v2 (perf rework vs the 345us baseline):
  - fp16 operands end-to-end (DMA volume halved; PE rate for fp16 equals
    fp32r at >=256-wide moving; DVE/ACT touch half the bytes).  All
    matmul accumulation stays fp32 in PSUM.
  - exp computed with a constant -4 bias so fp16 attention weights can
    never overflow (post-LN scores are bounded by sqrt(C)=11.3, and
    exp(11.3-4) = 1480 < fp16 max); the 1/L normalization cancels it.
  - host pretiles every dram tensor so each DMA reads contiguous
    per-partition lines (1.5-4KB bursts).
  - PE-transposes of q,k are software-pipelined one tile behind the qkv
    matmuls so the PE never waits on the LN/RoPE vector chain.
  - phase B runs a lookahead-2 softmax pipeline (scores two s-tiles
    ahead of the exp/mask consumers) and phase C (output projection) is
    interleaved one t-block behind attention so the y DMA-out streams
    during attention instead of tailing.
  - the 1.2us microcoded DVE reciprocals are replaced with
    reciprocal_approx_fast (~18 bits, single fast DVE op).
  - b_attn row is skipped when the bias is all zeros (it is in this
    problem); a rank-1 ones-outer-product matmul handles nonzero bias.
"""
import math
import os
import sys

sys.path.insert(0, "/opt/trn_rl_repo")

import numpy as np
from concourse import bacc, mybir, tile
from concourse import bass_utils

T, D, H, C = 2048, 2048, 16, 128
EPS = 1e-6
NCORES = 8
HPC = H // NCORES  # heads per core
DT = 16            # contraction tiles (no bias row; see EXPB)
F16 = mybir.dt.float16
F32 = mybir.dt.float32
F32R = mybir.dt.float32r
AF = mybir.ActivationFunctionType
ALU = mybir.AluOpType
AX = mybir.AxisListType

NT = T // 128      # 16 row tiles
NB = T // 512      # 4 big t-blocks
EXPB = -4.0        # constant exp bias; cancelled by 1/L

_NC_CACHE = {}
LAST_RESULT = None


def _build_program(with_bias):
    nc = bacc.Bacc("TRN2", target_bir_lowering=False, debug=False,
                   enable_asserts=True, num_devices=NCORES)

    xts = nc.dram_tensor("xts", [NT, 128, DT, 128], F16, kind="ExternalInput").ap()
    wts = nc.dram_tensor("wts", [DT, 128, 6 * C], F16, kind="ExternalInput").ap()
    rope = nc.dram_tensor("rope", [NT, 128, 1024], F16, kind="ExternalInput").ap()
    masks = nc.dram_tensor("masks", [128, 4 * 512], F16, kind="ExternalInput").ap()
    wpd = nc.dram_tensor("wpd", [128, HPC, D], F16, kind="ExternalInput").ap()
    onesrow = nc.dram_tensor("onesrow", [1, 128], F32R, kind="ExternalInput").ap()
    ident = nc.dram_tensor("ident", [128, 128], F16, kind="ExternalInput").ap()
    if with_bias:
        biasq = nc.dram_tensor("biasq", [1, 6 * C], F16, kind="ExternalInput").ap()
        ones1r = nc.dram_tensor("ones1r", [1, 128], F16, kind="ExternalInput").ap()
    y = nc.dram_tensor("y", [T, D], F16, kind="ExternalOutput").ap()

    sc = 1.0 / math.sqrt(C)

    with tile.TileContext(nc) as tc:
        with tc.tile_pool(name="res", bufs=1) as res:
            qT = res.tile([128, HPC, T], F16, tag="qT")        # [c, h, t]
            kT = res.tile([128, HPC, T], F16, tag="kT")
            vv = res.tile([128, HPC, NT, C], F16, tag="vv")    # [s, h, stile, c]
            ot = res.tile([128, HPC, T], F16, tag="ot")        # [c, h, t]
            w_sb = res.tile([128, DT, 6 * C], F16, tag="w_sb")
            masks_sb = res.tile([128, 4 * 512], F16, tag="masks")
            wp_sb = res.tile([128, HPC, D], F16, tag="wp")
            ones_c = res.tile([128, 1], F16, tag="ones_c")
            ones_r = res.tile([1, 128], F32R, tag="ones_r")
            id_sb = res.tile([128, 128], F16, tag="ident")
            zeros_c = res.tile([128, 1], F32, tag="zeros_c")
            negb_c = res.tile([128, 1], F32, tag="negb_c")
            eps_c = res.tile([128, 1], F32, tag="eps_c")
            if with_bias:
                bias_sb = res.tile([1, 6 * C], F16, tag="bias_sb")
                ones_1r = res.tile([1, 128], F16, tag="ones_1r")

            nc.gpsimd.memset(zeros_c[:], 0.0)
            nc.gpsimd.memset(negb_c[:], EXPB)
            nc.gpsimd.memset(eps_c[:], EPS)
            nc.gpsimd.memset(ones_c[:], 1.0)
            nc.sync.dma_start(ones_r[:], onesrow[:])
            nc.sync.dma_start(id_sb[:], ident[:])
            nc.sync.dma_start(masks_sb[:], masks[:])
            nc.sync.dma_start(wp_sb[:], wpd[:])
            if with_bias:
                nc.sync.dma_start(bias_sb[:], biasq[:])
                nc.sync.dma_start(ones_1r[:], ones1r[:])
            for dt in range(DT):
                nc.sync.dma_start(w_sb[:, dt, :], wts[dt])

            # =========== Phase A: QKV projection + LN + RoPE + transpose ===========
            with (
                tc.tile_pool(name="xcol", bufs=3) as xcolp,
                tc.tile_pool(name="ropep", bufs=3) as ropep,
                tc.tile_pool(name="qn", bufs=2) as qnp,
                tc.tile_pool(name="psA", bufs=3, space="PSUM") as psAp,
                tc.tile_pool(name="psB", bufs=2, space="PSUM") as psBp,
                tc.tile_pool(name="psT", bufs=2, space="PSUM") as psTp,
                tc.tile_pool(name="lnst", bufs=2) as lnstp,
                tc.tile_pool(name="sq", bufs=2) as sqp,
                tc.tile_pool(name="rot", bufs=2) as rotp,
            ):
                qn_prev = None

                def transpose_out(qn):
                    # PE transpose of the finished qn tile into qT/kT.
                    psT = psTp.tile([128, 4, 128], F16, tag="psT")
                    tt_, qn_t = qn
                    for i in range(4):
                        nc.tensor.transpose(
                            psT[:, i, :], qn_t[:, i * 128:(i + 1) * 128],
                            id_sb[:])
                    nc.scalar.activation(
                        qT[:, 0:2, tt_ * 128:(tt_ + 1) * 128],
                        psT[:, 0:2, :], AF.Copy)
                    nc.scalar.activation(
                        kT[:, 0:2, tt_ * 128:(tt_ + 1) * 128],
                        psT[:, 2:4, :], AF.Copy)

                for tt in range(NT):
                    xcol = xcolp.tile([128, DT, 128], F16, tag="xcol")
                    nc.sync.dma_start(xcol[:], xts[tt])
                    rc = ropep.tile([128, 1024], F16, tag="rope")
                    nc.sync.dma_start(rc[:], rope[tt])

                    psA = psAp.tile([128, 512], F32, tag="psA")
                    psB = psBp.tile([128, 256], F32, tag="psB")
                    for dt in range(DT):
                        nc.tensor.matmul(
                            psA[:], xcol[:, dt, :], w_sb[:, dt, 0:512],
                            start=(dt == 0),
                            stop=(dt == DT - 1 and not with_bias))
                        nc.tensor.matmul(
                            psB[:], xcol[:, dt, :], w_sb[:, dt, 512:768],
                            start=(dt == 0),
                            stop=(dt == DT - 1 and not with_bias))
                    if with_bias:
                        nc.tensor.matmul(
                            psA[:], ones_1r[:], bias_sb[:, 0:512],
                            start=False, stop=True)
                        nc.tensor.matmul(
                            psB[:], ones_1r[:], bias_sb[:, 512:768],
                            start=False, stop=True)

                    # transpose the PREVIOUS tile's qn while this tile's
                    # LN/RoPE chain runs -- keeps the PE stream unbroken.
                    if qn_prev is not None:
                        transpose_out(qn_prev)

                    nc.scalar.activation(
                        vv[:, 0:2, tt, :],
                        psB[:].rearrange("p (h c) -> p h c", h=2), AF.Copy)

                    st = lnstp.tile([128, 16], F32, tag="lnst")
                    # st cols: 0:4 sums, 4:8 sumsq, 8:12 rstd, 12:16 -mu*rstd
                    nc.vector.reduce_sum(
                        st[:, 0:4],
                        psA[:].rearrange("p (a b) -> p a b", a=4),
                        axis=AX.X)
                    for i in range(4):
                        sq = sqp.tile([128, 128], F32, tag="sq")
                        nc.scalar.activation(
                            sq[:], psA[:, i * 128:(i + 1) * 128], AF.Square,
                            bias=zeros_c[:], accum_out=st[:, 4 + i:5 + i])
                    mu = lnstp.tile([128, 8], F32, tag="mu")
                    nc.vector.tensor_scalar(
                        mu[:, 0:8], st[:, 0:8], 1.0 / C, None, op0=ALU.mult)
                    var = lnstp.tile([128, 4], F32, tag="var")
                    nc.vector.tensor_tensor(
                        var[:], mu[:, 0:4], mu[:, 0:4], op=ALU.mult)
                    nc.vector.tensor_tensor(
                        var[:], mu[:, 4:8], var[:], op=ALU.subtract)
                    nc.scalar.activation(var[:], var[:], AF.Sqrt,
                                         bias=eps_c[:])
                    nc.vector.reciprocal_approx_fast(st[:, 8:12], var[:])
                    nc.vector.tensor_tensor(
                        st[:, 12:16], mu[:, 0:4], st[:, 8:12], op=ALU.mult)
                    nc.vector.tensor_scalar(
                        st[:, 12:16], st[:, 12:16], -1.0, None, op0=ALU.mult)
                    qn = qnp.tile([128, 512], F16, tag="qn")
                    for i in range(4):
                        nc.vector.tensor_scalar(
                            qn[:, i * 128:(i + 1) * 128],
                            psA[:, i * 128:(i + 1) * 128],
                            st[:, 8 + i:9 + i], st[:, 12 + i:13 + i],
                            op0=ALU.mult, op1=ALU.add)
                    # RoPE: rot built on ACT (strided +-copies), combine on DVE
                    rot = rotp.tile([128, 512], F16, tag="rot")
                    qn3 = qn[:].rearrange("p (a b) -> p a b", b=2)
                    rot3 = rot[:].rearrange("p (a b) -> p a b", b=2)
                    nc.scalar.activation(
                        rot3[:, :, 0], qn3[:, :, 1], AF.Copy, scale=-1.0)
                    nc.scalar.activation(
                        rot3[:, :, 1], qn3[:, :, 0], AF.Copy)
                    nc.vector.tensor_tensor(
                        qn[:], qn[:], rc[:, 0:512], op=ALU.mult)
                    nc.vector.tensor_tensor(
                        rot[:], rot[:], rc[:, 512:1024], op=ALU.mult)
                    nc.vector.tensor_tensor(qn[:], qn[:], rot[:], op=ALU.add)
                    qn_prev = (tt, qn)

                transpose_out(qn_prev)

            # =========== Phase B+C: attention + output projection ===========
            with (
                tc.tile_pool(name="psS", bufs=2, space="PSUM") as psSp,
                tc.tile_pool(name="psO", bufs=2, space="PSUM") as psOp,
                tc.tile_pool(name="psL", bufs=1, space="PSUM") as psLp,
                tc.tile_pool(name="psBC", bufs=1, space="PSUM") as psBCp,
                tc.tile_pool(name="psY", bufs=2, space="PSUM") as psYp,
                tc.tile_pool(name="aT", bufs=4) as aTp,
                tc.tile_pool(name="bsm", bufs=2) as bsmp,
                tc.tile_pool(name="ysb", bufs=2) as ysbp,
            ):
                def attn_block(h, tb):
                    S = 4 * (tb + 1)
                    qTs = qT[:, h, tb * 512:(tb + 1) * 512]
                    Lps = psLp.tile([1, 512], F32, tag="psL")
                    Ops = psOp.tile([128, 512], F32, tag="psO")
                    st_ps = [None] * S

                    def emit_st(s):
                        stp = psSp.tile([128, 512], F32, tag="psS")
                        nc.tensor.matmul(
                            stp[:], kT[:, h, s * 128:(s + 1) * 128], qTs,
                            start=True, stop=True)
                        st_ps[s] = stp

                    emit_st(0)
                    if S > 1:
                        emit_st(1)
                    for s in range(S):
                        a = aTp.tile([128, 512], F16, tag="aT")
                        nc.scalar.activation(
                            a[:], st_ps[s][:], AF.Exp,
                            bias=negb_c[:], scale=sc)
                        st_ps[s] = None
                        if s >= 4 * tb:
                            j = s - 4 * tb
                            nc.vector.tensor_tensor(
                                a[:], a[:],
                                masks_sb[:, j * 512:(j + 1) * 512],
                                op=ALU.mult)
                        if s + 2 < S:
                            emit_st(s + 2)
                        nc.tensor.matmul(
                            Lps[:], ones_c[:], a[:],
                            start=(s == 0), stop=(s == S - 1))
                        nc.tensor.matmul(
                            Ops[:], vv[:, h, s, :], a[:],
                            start=(s == 0), stop=(s == S - 1))
                    recL = bsmp.tile([1, 512], F32, tag="recL")
                    nc.vector.reciprocal_approx_fast(recL[:], Lps[:])
                    recLr = bsmp.tile([1, 512], F32R, tag="recLr")
                    nc.scalar.activation(recLr[:], recL[:], AF.Copy)
                    bc = psBCp.tile([128, 512], F32, tag="psBC")
                    nc.tensor.matmul(bc[:], ones_r[:], recLr[:],
                                     start=True, stop=True)
                    bcs = bsmp.tile([128, 512], F32, tag="bcs")
                    nc.scalar.activation(bcs[:], bc[:], AF.Copy)
                    nc.vector.tensor_tensor(
                        ot[:, h, tb * 512:(tb + 1) * 512], Ops[:],
                        bcs[:], op=ALU.mult)

                def proj_block(tb):
                    # y rows [tb*512, (tb+1)*512) need ot cols of this tb only
                    for ttt in range(4 * tb, 4 * tb + 4):
                        ysb = ysbp.tile([128, 4, 512], F16, tag="ysb")
                        for db in range(NB):
                            yps = psYp.tile([128, 512], F32, tag="psY")
                            for h in range(HPC):
                                nc.tensor.matmul(
                                    yps[:],
                                    ot[:, h, ttt * 128:(ttt + 1) * 128],
                                    wp_sb[:, h, db * 512:(db + 1) * 512],
                                    start=(h == 0), stop=(h == HPC - 1))
                            if db % 2 == 0:
                                nc.scalar.activation(
                                    ysb[:, db, :], yps[:], AF.Copy)
                            else:
                                nc.vector.tensor_copy(ysb[:, db, :], yps[:])
                        nc.sync.dma_start(
                            y[ttt * 128:(ttt + 1) * 128, :],
                            ysb[:].rearrange("p a b -> p (a b)"))

                for tb in range(NB):
                    attn_block(0, tb)
                    attn_block(1, tb)
                    if tb > 0:
                        proj_block(tb - 1)
                proj_block(NB - 1)

    nc.compile()
    return nc


def _host_prep(x, W_attn, b_attn, W_proj, q_ln_w, k_ln_w):
    f = np.float32
    h16 = np.float16

    # x pretiled: xts[tt, p, a, j] = x[tt*128+j, a*128+p]
    x4 = x.reshape(NT, 128, DT, 128)          # [tt, j, a, p]
    xts = np.ascontiguousarray(
        x4.transpose(0, 3, 2, 1).astype(h16))  # [tt, p, a, j]

    inv = (1.0 / (10000.0 ** (np.arange(0, C, 2, dtype=f) / C))).astype(f)
    freqs = np.arange(T, dtype=f)[:, None] * inv[None, :]
    sin = np.repeat(np.sin(freqs), 2, axis=1).astype(f)
    cos = np.repeat(np.cos(freqs), 2, axis=1).astype(f)
    part = np.arange(C) ^ 1
    cos_q = cos * q_ln_w[None, :]
    sin_q = sin * q_ln_w[None, part]
    cos_k = cos * k_ln_w[None, :]
    sin_k = sin * k_ln_w[None, part]
    ropecos = np.concatenate([cos_q, cos_q, cos_k, cos_k], axis=1)
    ropesin = np.concatenate([sin_q, sin_q, sin_k, sin_k], axis=1)
    ropetab = np.ascontiguousarray(
        np.concatenate([ropecos, ropesin], axis=1)
        .reshape(NT, 128, 1024).astype(h16))

    ss = np.arange(128)[:, None]
    ttm = np.arange(512)[None, :]
    masks = np.ascontiguousarray(np.concatenate(
        [(j * 128 + ss <= ttm).astype(h16) for j in range(4)], axis=1))

    with_bias = bool(np.any(b_attn != 0.0))

    shared = dict(xts=xts, rope=ropetab, masks=masks,
                  onesrow=np.ones((1, 128), f),
                  ident=np.eye(128, dtype=h16))
    if with_bias:
        shared["ones1r"] = np.ones((1, 128), h16)

    in_maps = []
    for c in range(NCORES):
        h0, h1 = HPC * c, HPC * c + 1
        rows = np.concatenate([
            np.arange(h0 * C, (h0 + 1) * C),
            np.arange(h1 * C, (h1 + 1) * C),
            D + np.arange(h0 * C, (h0 + 1) * C),
            D + np.arange(h1 * C, (h1 + 1) * C),
            2 * D + np.arange(h0 * C, (h0 + 1) * C),
            2 * D + np.arange(h1 * C, (h1 + 1) * C),
        ])
        wq = W_attn[rows].T                    # [D, 768]
        wts = np.ascontiguousarray(
            wq.reshape(DT, 128, 6 * C).astype(h16))
        wpc = np.stack(
            [W_proj[:, h0 * C:(h0 + 1) * C].T,
             W_proj[:, h1 * C:(h1 + 1) * C].T], axis=0)  # [2, 128, D]
        wpd = np.ascontiguousarray(wpc.transpose(1, 0, 2).astype(h16))
        m = dict(shared)
        m["wts"] = wts
        m["wpd"] = wpd
        if with_bias:
            m["biasq"] = np.ascontiguousarray(b_attn[rows][None, :]).astype(h16)
        in_maps.append(m)
    return in_maps, with_bias


def kernel(x, W_attn, b_attn, W_proj, b_proj, q_ln_w, k_ln_w):
    global LAST_RESULT
    f = np.float32
    x = np.ascontiguousarray(np.asarray(x, f))
    W_attn = np.ascontiguousarray(np.asarray(W_attn, f))
    b_attn = np.ascontiguousarray(np.asarray(b_attn, f))
    W_proj = np.ascontiguousarray(np.asarray(W_proj, f))
    b_proj = np.ascontiguousarray(np.asarray(b_proj, f))
    q_ln_w = np.ascontiguousarray(np.asarray(q_ln_w, f))
    k_ln_w = np.ascontiguousarray(np.asarray(k_ln_w, f))

    in_maps, with_bias = _host_prep(x, W_attn, b_attn, W_proj, q_ln_w, k_ln_w)
    if with_bias not in _NC_CACHE:
        _NC_CACHE[with_bias] = _build_program(with_bias)
    nc = _NC_CACHE[with_bias]

    res = bass_utils.run_bass_kernel_spmd(
        nc, in_maps, core_ids=list(range(NCORES)),
        trace=bool(os.environ.get("BASS_TRACE")))
    LAST_RESULT = res

    y = np.zeros((T, D), np.float32)
    for rmap in res.results:
        y += rmap["y"].astype(np.float32)
    y += b_proj[None, :]
    return y


# revision 5
# speedup vs baseline: 1.4602x; 1.0381x over previous
"""nn_CausalSelfAttention_88854283420050 — Bass/Tile kernel for 8 trn2 cores.

Sharding: tensor-parallel over heads (H=16 -> 2 heads per core).
Each core computes, for its 2 heads: the qkv projection (columns of
c_attn), per-head LayerNorm + RoPE, causal attention, and a partial
output projection y_c = O_heads @ W_proj[:, head cols].T.  The host
sums the 8 partial projections (row-parallel c_proj) and adds b_proj.

v3 highlights (345us baseline -> 245us -> this):
  - fp16 operands end-to-end (fp32 accumulation in PSUM); exp carries a
    constant -4 bias so fp16 attention weights cannot overflow (post-LN
    scores are bounded by sqrt(C); the 1/L normalization cancels it).
  - per-head channel sums ride as 4 extra columns of the qkv matmul, so
    the LN mean comes out of the PE for free.
  - causal structure exploited at 128-col granularity: on diagonal
    s-tiles the scores/exp/L/O matmuls all run only on the nonzero
    column range, and the mask multiply shrinks to one shared
    [128,128] triangle.
  - engine balance: RoPE swap-copies and bcs broadcast drain on DVE,
    squares/copies on ACT, phase-C psum drains alternate ACT/DVE.
  - PE-transposes of q,k pipelined one tile behind the qkv matmuls;
    phase C interleaved one t-block behind attention; lookahead-2
    softmax pipeline; startup DMAs ordered so the first matmul starts
    as soon as w-chunk 0 and x-tile 0 land.
"""
import math
import os
import sys

sys.path.insert(0, "/opt/trn_rl_repo")

import numpy as np
from concourse import bacc, mybir, tile
from concourse import bass_utils

T, D, H, C = 2048, 2048, 16, 128
EPS = 1e-6
NCORES = 8
HPC = H // NCORES  # heads per core
DT = 16            # contraction tiles (no bias row)
F16 = mybir.dt.float16
F32 = mybir.dt.float32
F32R = mybir.dt.float32r
AF = mybir.ActivationFunctionType
ALU = mybir.AluOpType
AX = mybir.AxisListType

NT = T // 128      # 16 row tiles
NB = T // 512      # 4 big t-blocks
WQ = 6 * C + 4     # qkv weight cols + 4 per-head sum cols
EXPB = -4.0        # constant exp bias; cancelled by 1/L

_NC_CACHE = {}
LAST_RESULT = None


def _build_program(with_bias):
    nc = bacc.Bacc("TRN2", target_bir_lowering=False, debug=False,
                   enable_asserts=True, num_devices=NCORES)

    xts = nc.dram_tensor("xts", [NT, 128, DT, 128], F16, kind="ExternalInput").ap()
    wts = nc.dram_tensor("wts", [DT, 128, WQ], F16, kind="ExternalInput").ap()
    rope = nc.dram_tensor("rope", [NT, 128, 1024], F16, kind="ExternalInput").ap()
    masks = nc.dram_tensor("masks", [128, 128], F16, kind="ExternalInput").ap()
    wpd = nc.dram_tensor("wpd", [128, HPC, D], F16, kind="ExternalInput").ap()
    onesrow = nc.dram_tensor("onesrow", [1, 128], F32R, kind="ExternalInput").ap()
    ident = nc.dram_tensor("ident", [128, 128], F16, kind="ExternalInput").ap()
    if with_bias:
        biasq = nc.dram_tensor("biasq", [1, WQ], F16, kind="ExternalInput").ap()
        ones1r = nc.dram_tensor("ones1r", [1, 128], F16, kind="ExternalInput").ap()
    y = nc.dram_tensor("y", [T, D], F16, kind="ExternalOutput").ap()

    sc = 1.0 / math.sqrt(C)

    with tile.TileContext(nc) as tc:
        with tc.tile_pool(name="res", bufs=1) as res:
            qT = res.tile([128, HPC, T], F16, tag="qT")        # [c, h, t]
            kT = res.tile([128, HPC, T], F16, tag="kT")
            vv = res.tile([128, HPC, NT, C], F16, tag="vv")    # [s, h, stile, c]
            ot = res.tile([128, HPC, T], F16, tag="ot")        # [c, h, t]
            w_sb = res.tile([128, DT, WQ], F16, tag="w_sb")
            masks_sb = res.tile([128, 128], F16, tag="masks")
            wp_sb = res.tile([128, HPC, D], F16, tag="wp")
            ones_c = res.tile([128, 1], F16, tag="ones_c")
            ones_r = res.tile([1, 128], F32R, tag="ones_r")
            id_sb = res.tile([128, 128], F16, tag="ident")
            zeros_c = res.tile([128, 1], F32, tag="zeros_c")
            negb_c = res.tile([128, 1], F32, tag="negb_c")
            eps_c = res.tile([128, 1], F32, tag="eps_c")
            if with_bias:
                bias_sb = res.tile([1, WQ], F16, tag="bias_sb")
                ones_1r = res.tile([1, 128], F16, tag="ones_1r")

            nc.gpsimd.memset(zeros_c[:], 0.0)
            nc.gpsimd.memset(negb_c[:], EXPB)
            nc.gpsimd.memset(eps_c[:], EPS)
            nc.gpsimd.memset(ones_c[:], 1.0)

            # =========== Phase A: QKV projection + LN + RoPE + transpose ===========
            with (
                tc.tile_pool(name="xcol", bufs=3) as xcolp,
                tc.tile_pool(name="ropep", bufs=3) as ropep,
                tc.tile_pool(name="qn", bufs=2) as qnp,
                tc.tile_pool(name="psA", bufs=3, space="PSUM") as psAp,
                tc.tile_pool(name="psB", bufs=2, space="PSUM") as psBp,
                tc.tile_pool(name="psT", bufs=2, space="PSUM") as psTp,
                tc.tile_pool(name="lnst", bufs=2) as lnstp,
                tc.tile_pool(name="sq", bufs=2) as sqp,
                tc.tile_pool(name="rot", bufs=2) as rotp,
            ):
                qn_prev = None

                def transpose_out(qn):
                    # PE transpose of the finished qn tile into qT/kT.
                    psT = psTp.tile([128, 4, 128], F16, tag="psT")
                    tt_, qn_t = qn
                    for i in range(4):
                        nc.tensor.transpose(
                            psT[:, i, :], qn_t[:, i * 128:(i + 1) * 128],
                            id_sb[:])
                    nc.scalar.activation(
                        qT[:, 0:2, tt_ * 128:(tt_ + 1) * 128],
                        psT[:, 0:2, :], AF.Copy)
                    nc.scalar.activation(
                        kT[:, 0:2, tt_ * 128:(tt_ + 1) * 128],
                        psT[:, 2:4, :], AF.Copy)

                for tt in range(NT):
                    xcol = xcolp.tile([128, DT, 128], F16, tag="xcol")
                    rc = ropep.tile([128, 1024], F16, tag="rope")
                    if tt == 0:
                        # first w chunk, then x/rope so matmuls start early
                        nc.sync.dma_start(w_sb[:, 0, :], wts[0])
                        nc.sync.dma_start(xcol[:], xts[tt])
                        nc.sync.dma_start(rc[:], rope[tt])
                        for dt in range(1, DT):
                            nc.sync.dma_start(w_sb[:, dt, :], wts[dt])
                        nc.sync.dma_start(id_sb[:], ident[:])
                        nc.sync.dma_start(ones_r[:], onesrow[:])
                        if with_bias:
                            nc.sync.dma_start(bias_sb[:], biasq[:])
                            nc.sync.dma_start(ones_1r[:], ones1r[:])
                    else:
                        nc.sync.dma_start(xcol[:], xts[tt])
                        nc.sync.dma_start(rc[:], rope[tt])
                    if tt == 2:
                        nc.sync.dma_start(masks_sb[:], masks[:])
                        nc.sync.dma_start(wp_sb[:], wpd[:])

                    psA = psAp.tile([128, 512], F32, tag="psA")
                    psB = psBp.tile([128, 260], F32, tag="psB")
                    for dt in range(DT):
                        nc.tensor.matmul(
                            psA[:], xcol[:, dt, :], w_sb[:, dt, 0:512],
                            start=(dt == 0),
                            stop=(dt == DT - 1 and not with_bias))
                        nc.tensor.matmul(
                            psB[:], xcol[:, dt, :], w_sb[:, dt, 512:772],
                            start=(dt == 0),
                            stop=(dt == DT - 1 and not with_bias))
                    if with_bias:
                        nc.tensor.matmul(
                            psA[:], ones_1r[:], bias_sb[:, 0:512],
                            start=False, stop=True)
                        nc.tensor.matmul(
                            psB[:], ones_1r[:], bias_sb[:, 512:772],
                            start=False, stop=True)

                    # transpose the PREVIOUS tile's qn while this tile's
                    # LN/RoPE chain runs -- keeps the PE stream unbroken.
                    if qn_prev is not None:
                        transpose_out(qn_prev)

                    nc.scalar.activation(
                        vv[:, 0:2, tt, :],
                        psB[:, 0:256].rearrange("p (h c) -> p h c", h=2),
                        AF.Copy)

                    st = lnstp.tile([128, 16], F32, tag="lnst")
                    # st cols: 0:4 mu, 4:8 sumsq, 8:12 rstd, 12:16 -mu*rstd
                    nc.vector.tensor_scalar(
                        st[:, 0:4], psB[:, 256:260], 1.0 / C, None,
                        op0=ALU.mult)
                    for i in range(4):
                        sq = sqp.tile([128, 128], F32, tag="sq")
                        nc.scalar.activation(
                            sq[:], psA[:, i * 128:(i + 1) * 128], AF.Square,
                            bias=zeros_c[:], accum_out=st[:, 4 + i:5 + i])
                    var = lnstp.tile([128, 4], F32, tag="var")
                    nc.vector.tensor_scalar(
                        var[:], st[:, 4:8], 1.0 / C, None, op0=ALU.mult)
                    mu2 = lnstp.tile([128, 4], F32, tag="mu2")
                    nc.vector.tensor_tensor(
                        mu2[:], st[:, 0:4], st[:, 0:4], op=ALU.mult)
                    nc.vector.tensor_tensor(
                        var[:], var[:], mu2[:], op=ALU.subtract)
                    nc.scalar.activation(var[:], var[:], AF.Sqrt,
                                         bias=eps_c[:])
                    nc.vector.reciprocal_approx_fast(st[:, 8:12], var[:])
                    nc.vector.tensor_tensor(
                        st[:, 12:16], st[:, 0:4], st[:, 8:12], op=ALU.mult)
                    nc.vector.tensor_scalar(
                        st[:, 12:16], st[:, 12:16], -1.0, None, op0=ALU.mult)
                    qn = qnp.tile([128, 512], F16, tag="qn")
                    for i in range(4):
                        nc.vector.tensor_scalar(
                            qn[:, i * 128:(i + 1) * 128],
                            psA[:, i * 128:(i + 1) * 128],
                            st[:, 8 + i:9 + i], st[:, 12 + i:13 + i],
                            op0=ALU.mult, op1=ALU.add)
                    # RoPE: swap-copies on DVE, combines on DVE
                    rot = rotp.tile([128, 512], F16, tag="rot")
                    qn3 = qn[:].rearrange("p (a b) -> p a b", b=2)
                    rot3 = rot[:].rearrange("p (a b) -> p a b", b=2)
                    nc.vector.tensor_scalar(
                        rot3[:, :, 0], qn3[:, :, 1], -1.0, None, op0=ALU.mult)
                    nc.vector.tensor_copy(rot3[:, :, 1], qn3[:, :, 0])
                    nc.vector.tensor_tensor(
                        qn[:], qn[:], rc[:, 0:512], op=ALU.mult)
                    nc.vector.tensor_tensor(
                        rot[:], rot[:], rc[:, 512:1024], op=ALU.mult)
                    nc.vector.tensor_tensor(qn[:], qn[:], rot[:], op=ALU.add)
                    qn_prev = (tt, qn)

                transpose_out(qn_prev)

            # =========== Phase B+C: attention + output projection ===========
            with (
                tc.tile_pool(name="psS", bufs=2, space="PSUM") as psSp,
                tc.tile_pool(name="psO", bufs=2, space="PSUM") as psOp,
                tc.tile_pool(name="psL", bufs=1, space="PSUM") as psLp,
                tc.tile_pool(name="psBC", bufs=1, space="PSUM") as psBCp,
                tc.tile_pool(name="psY", bufs=2, space="PSUM") as psYp,
                tc.tile_pool(name="aT", bufs=4) as aTp,
                tc.tile_pool(name="bsm", bufs=2) as bsmp,
                tc.tile_pool(name="ysb", bufs=2) as ysbp,
            ):
                def attn_block(h, tb):
                    S = 4 * (tb + 1)
                    qTs = qT[:, h, tb * 512:(tb + 1) * 512]

                    def lo_of(s):
                        # first nonzero column of s-tile s in this t-block
                        return (s - 4 * tb) * 128 if s >= 4 * tb else 0

                    Lps = psLp.tile([1, 512], F32, tag="psL")
                    Ops = psOp.tile([128, 512], F32, tag="psO")
                    st_ps = [None] * S

                    def emit_st(s):
                        stp = psSp.tile([128, 512], F32, tag="psS")
                        lo = lo_of(s)
                        nc.tensor.matmul(
                            stp[:, lo:512],
                            kT[:, h, s * 128:(s + 1) * 128], qTs[:, lo:512],
                            start=True, stop=True)
                        st_ps[s] = stp

                    emit_st(0)
                    if S > 1:
                        emit_st(1)
                    for s in range(S):
                        lo = lo_of(s)
                        a = aTp.tile([128, 512], F16, tag="aT")
                        nc.scalar.activation(
                            a[:, lo:512], st_ps[s][:, lo:512], AF.Exp,
                            bias=negb_c[:], scale=sc)
                        st_ps[s] = None
                        if s >= 4 * tb:
                            # only the [128,128] triangle block needs masking
                            nc.vector.tensor_tensor(
                                a[:, lo:lo + 128], a[:, lo:lo + 128],
                                masks_sb[:], op=ALU.mult)
                        if s + 2 < S:
                            emit_st(s + 2)
                        nc.tensor.matmul(
                            Lps[:, lo:512], ones_c[:], a[:, lo:512],
                            start=(s == 0), stop=(s == S - 1))
                        nc.tensor.matmul(
                            Ops[:, lo:512], vv[:, h, s, :], a[:, lo:512],
                            start=(s == 0), stop=(s == S - 1))
                    recL = bsmp.tile([1, 512], F32, tag="recL")
                    nc.vector.reciprocal_approx_fast(recL[:], Lps[:])
                    recLr = bsmp.tile([1, 512], F32R, tag="recLr")
                    nc.scalar.activation(recLr[:], recL[:], AF.Copy)
                    bc = psBCp.tile([128, 512], F32, tag="psBC")
                    nc.tensor.matmul(bc[:], ones_r[:], recLr[:],
                                     start=True, stop=True)
                    bcs = bsmp.tile([128, 512], F32, tag="bcs")
                    nc.vector.tensor_copy(bcs[:], bc[:])
                    nc.vector.tensor_tensor(
                        ot[:, h, tb * 512:(tb + 1) * 512], Ops[:],
                        bcs[:], op=ALU.mult)

                def proj_block(tb):
                    # y rows [tb*512, (tb+1)*512) need ot cols of this tb only
                    for ttt in range(4 * tb, 4 * tb + 4):
                        ysb = ysbp.tile([128, 4, 512], F16, tag="ysb")
                        for db in range(NB):
                            yps = psYp.tile([128, 512], F32, tag="psY")
                            for h in range(HPC):
                                nc.tensor.matmul(
                                    yps[:],
                                    ot[:, h, ttt * 128:(ttt + 1) * 128],
                                    wp_sb[:, h, db * 512:(db + 1) * 512],
                                    start=(h == 0), stop=(h == HPC - 1))
                            if db % 2 == 0:
                                nc.scalar.activation(
                                    ysb[:, db, :], yps[:], AF.Copy)
                            else:
                                nc.vector.tensor_copy(ysb[:, db, :], yps[:])
                        nc.sync.dma_start(
                            y[ttt * 128:(ttt + 1) * 128, :],
                            ysb[:].rearrange("p a b -> p (a b)"))

                for tb in range(NB):
                    attn_block(0, tb)
                    attn_block(1, tb)
                    if tb > 0:
                        proj_block(tb - 1)
                proj_block(NB - 1)

    nc.compile()
    return nc


def _host_prep(x, W_attn, b_attn, W_proj, q_ln_w, k_ln_w):
    f = np.float32
    h16 = np.float16

    # x pretiled: xts[tt, p, a, j] = x[tt*128+j, a*128+p]
    x4 = x.reshape(NT, 128, DT, 128)          # [tt, j, a, p]
    xts = np.ascontiguousarray(
        x4.transpose(0, 3, 2, 1).astype(h16))  # [tt, p, a, j]

    inv = (1.0 / (10000.0 ** (np.arange(0, C, 2, dtype=f) / C))).astype(f)
    freqs = np.arange(T, dtype=f)[:, None] * inv[None, :]
    sin = np.repeat(np.sin(freqs), 2, axis=1).astype(f)
    cos = np.repeat(np.cos(freqs), 2, axis=1).astype(f)
    part = np.arange(C) ^ 1
    cos_q = cos * q_ln_w[None, :]
    sin_q = sin * q_ln_w[None, part]
    cos_k = cos * k_ln_w[None, :]
    sin_k = sin * k_ln_w[None, part]
    ropecos = np.concatenate([cos_q, cos_q, cos_k, cos_k], axis=1)
    ropesin = np.concatenate([sin_q, sin_q, sin_k, sin_k], axis=1)
    ropetab = np.ascontiguousarray(
        np.concatenate([ropecos, ropesin], axis=1)
        .reshape(NT, 128, 1024).astype(h16))

    ss = np.arange(128)[:, None]
    ttm = np.arange(128)[None, :]
    masks = np.ascontiguousarray((ss <= ttm).astype(h16))

    with_bias = bool(np.any(b_attn != 0.0))

    shared = dict(xts=xts, rope=ropetab, masks=masks,
                  onesrow=np.ones((1, 128), f),
                  ident=np.eye(128, dtype=h16))
    if with_bias:
        shared["ones1r"] = np.ones((1, 128), h16)

    in_maps = []
    for c in range(NCORES):
        h0, h1 = HPC * c, HPC * c + 1
        rows = np.concatenate([
            np.arange(h0 * C, (h0 + 1) * C),
            np.arange(h1 * C, (h1 + 1) * C),
            D + np.arange(h0 * C, (h0 + 1) * C),
            D + np.arange(h1 * C, (h1 + 1) * C),
            2 * D + np.arange(h0 * C, (h0 + 1) * C),
            2 * D + np.arange(h1 * C, (h1 + 1) * C),
        ])
        wq = W_attn[rows].T                    # [D, 768]
        # 4 extra columns: per-head channel sums of the q/k blocks so the
        # LN mean comes out of the qkv matmul directly.
        wsum = wq[:, 0:512].reshape(D, 4, 128).sum(axis=2)   # [D, 4]
        wqa = np.concatenate([wq, wsum], axis=1)             # [D, 772]
        wts = np.ascontiguousarray(
            wqa.reshape(DT, 128, WQ).astype(h16))
        wpc = np.stack(
            [W_proj[:, h0 * C:(h0 + 1) * C].T,
             W_proj[:, h1 * C:(h1 + 1) * C].T], axis=0)  # [2, 128, D]
        wpd = np.ascontiguousarray(wpc.transpose(1, 0, 2).astype(h16))
        m = dict(shared)
        m["wts"] = wts
        m["wpd"] = wpd
        if with_bias:
            ba = b_attn[rows]
            bs = ba[0:512].reshape(4, 128).sum(axis=1)
            m["biasq"] = np.ascontiguousarray(
                np.concatenate([ba, bs])[None, :]).astype(h16)
        in_maps.append(m)
    return in_maps, with_bias


def kernel(x, W_attn, b_attn, W_proj, b_proj, q_ln_w, k_ln_w):
    global LAST_RESULT
    f = np.float32
    x = np.ascontiguousarray(np.asarray(x, f))
    W_attn = np.ascontiguousarray(np.asarray(W_attn, f))
    b_attn = np.ascontiguousarray(np.asarray(b_attn, f))
    W_proj = np.ascontiguousarray(np.asarray(W_proj, f))
    b_proj = np.ascontiguousarray(np.asarray(b_proj, f))
    q_ln_w = np.ascontiguousarray(np.asarray(q_ln_w, f))
    k_ln_w = np.ascontiguousarray(np.asarray(k_ln_w, f))

    in_maps, with_bias = _host_prep(x, W_attn, b_attn, W_proj, q_ln_w, k_ln_w)
    if with_bias not in _NC_CACHE:
        _NC_CACHE[with_bias] = _build_program(with_bias)
    nc = _NC_CACHE[with_bias]

    res = bass_utils.run_bass_kernel_spmd(
        nc, in_maps, core_ids=list(range(NCORES)),
        trace=bool(os.environ.get("BASS_TRACE")))
    LAST_RESULT = res

    y = np.zeros((T, D), np.float32)
    for rmap in res.results:
        y += rmap["y"].astype(np.float32)
    y += b_proj[None, :]
    return y


# revision 7
# speedup vs baseline: 1.4745x; 1.0097x over previous
"""nn_CausalSelfAttention_88854283420050 — Bass/Tile kernel for 8 trn2 cores.

Sharding: tensor-parallel over heads (H=16 -> 2 heads per core).
Each core computes, for its 2 heads: the qkv projection (columns of
c_attn), per-head LayerNorm + RoPE, causal attention, and a partial
output projection y_c = O_heads @ W_proj[:, head cols].T.  The host
sums the 8 partial projections (row-parallel c_proj) and adds b_proj.

v3 highlights (345us baseline -> 245us -> this):
  - fp16 operands end-to-end (fp32 accumulation in PSUM); exp carries a
    constant -4 bias so fp16 attention weights cannot overflow (post-LN
    scores are bounded by sqrt(C); the 1/L normalization cancels it).
  - per-head channel sums ride as 4 extra columns of the qkv matmul, so
    the LN mean comes out of the PE for free.
  - causal structure exploited at 128-col granularity: on diagonal
    s-tiles the scores/exp/L/O matmuls all run only on the nonzero
    column range, and the mask multiply shrinks to one shared
    [128,128] triangle.
  - engine balance: RoPE swap-copies and bcs broadcast drain on DVE,
    squares/copies on ACT, phase-C psum drains alternate ACT/DVE.
  - PE-transposes of q,k pipelined one tile behind the qkv matmuls;
    phase C interleaved one t-block behind attention; lookahead-2
    softmax pipeline; startup DMAs ordered so the first matmul starts
    as soon as w-chunk 0 and x-tile 0 land.
"""
import math
import os
import sys

sys.path.insert(0, "/opt/trn_rl_repo")

import numpy as np
from concourse import bacc, mybir, tile
from concourse import bass_utils

T, D, H, C = 2048, 2048, 16, 128
EPS = 1e-6
NCORES = 8
HPC = H // NCORES  # heads per core
DT = 16            # contraction tiles (no bias row)
F16 = mybir.dt.float16
F32 = mybir.dt.float32
F32R = mybir.dt.float32r
AF = mybir.ActivationFunctionType
ALU = mybir.AluOpType
AX = mybir.AxisListType

NT = T // 128      # 16 row tiles
NB = T // 512      # 4 big t-blocks
WQ = 6 * C + 4     # qkv weight cols + 4 per-head sum cols
EXPB = -4.0        # constant exp bias; cancelled by 1/L

_NC_CACHE = {}
LAST_RESULT = None


def _build_program(with_bias):
    nc = bacc.Bacc("TRN2", target_bir_lowering=False, debug=False,
                   enable_asserts=True, num_devices=NCORES)

    xts = nc.dram_tensor("xts", [NT, 128, DT, 128], F16, kind="ExternalInput").ap()
    wts = nc.dram_tensor("wts", [DT, 128, WQ], F16, kind="ExternalInput").ap()
    rope = nc.dram_tensor("rope", [NT, 128, 1024], F16, kind="ExternalInput").ap()
    masks = nc.dram_tensor("masks", [128, 128], F16, kind="ExternalInput").ap()
    wpd = nc.dram_tensor("wpd", [128, HPC, D], F16, kind="ExternalInput").ap()
    onesrow = nc.dram_tensor("onesrow", [1, 128], F32R, kind="ExternalInput").ap()
    ident = nc.dram_tensor("ident", [128, 128], F16, kind="ExternalInput").ap()
    if with_bias:
        biasq = nc.dram_tensor("biasq", [1, WQ], F16, kind="ExternalInput").ap()
        ones1r = nc.dram_tensor("ones1r", [1, 128], F16, kind="ExternalInput").ap()
    y = nc.dram_tensor("y", [T, D], F16, kind="ExternalOutput").ap()

    sc = 1.0 / math.sqrt(C)

    with tile.TileContext(nc) as tc:
        with tc.tile_pool(name="res", bufs=1) as res:
            qT = res.tile([128, HPC, T], F16, tag="qT")        # [c, h, t]
            kT = res.tile([128, HPC, T], F16, tag="kT")
            vv = res.tile([128, HPC, NT, C], F16, tag="vv")    # [s, h, stile, c]
            ot = res.tile([128, HPC, T], F16, tag="ot")        # [c, h, t]
            w_sb = res.tile([128, DT, WQ], F16, tag="w_sb")
            masks_sb = res.tile([128, 128], F16, tag="masks")
            wp_sb = res.tile([128, HPC, D], F16, tag="wp")
            ones_c = res.tile([128, 1], F16, tag="ones_c")
            ones_r = res.tile([1, 128], F32R, tag="ones_r")
            id_sb = res.tile([128, 128], F16, tag="ident")
            zeros_c = res.tile([128, 1], F32, tag="zeros_c")
            negb_c = res.tile([128, 1], F32, tag="negb_c")
            eps_c = res.tile([128, 1], F32, tag="eps_c")
            if with_bias:
                bias_sb = res.tile([1, WQ], F16, tag="bias_sb")
                ones_1r = res.tile([1, 128], F16, tag="ones_1r")

            nc.gpsimd.memset(zeros_c[:], 0.0)
            nc.gpsimd.memset(negb_c[:], EXPB)
            nc.gpsimd.memset(eps_c[:], EPS)
            nc.gpsimd.memset(ones_c[:], 1.0)

            # =========== Phase A: QKV projection + LN + RoPE + transpose ===========
            with (
                tc.tile_pool(name="xcol", bufs=3) as xcolp,
                tc.tile_pool(name="ropep", bufs=3) as ropep,
                tc.tile_pool(name="qn", bufs=2) as qnp,
                tc.tile_pool(name="psA", bufs=3, space="PSUM") as psAp,
                tc.tile_pool(name="psB", bufs=2, space="PSUM") as psBp,
                tc.tile_pool(name="psT", bufs=2, space="PSUM") as psTp,
                tc.tile_pool(name="lnst", bufs=2) as lnstp,
                tc.tile_pool(name="sq", bufs=2) as sqp,
                tc.tile_pool(name="rot", bufs=2) as rotp,
            ):
                qn_prev = None

                def transpose_out(qn):
                    # PE transpose of the finished qn tile into qT/kT.
                    psT = psTp.tile([128, 4, 128], F16, tag="psT")
                    tt_, qn_t = qn
                    for i in range(4):
                        nc.tensor.transpose(
                            psT[:, i, :], qn_t[:, i * 128:(i + 1) * 128],
                            id_sb[:])
                    nc.scalar.activation(
                        qT[:, 0:2, tt_ * 128:(tt_ + 1) * 128],
                        psT[:, 0:2, :], AF.Copy)
                    nc.scalar.activation(
                        kT[:, 0:2, tt_ * 128:(tt_ + 1) * 128],
                        psT[:, 2:4, :], AF.Copy)

                for tt in range(NT):
                    xcol = xcolp.tile([128, DT, 128], F16, tag="xcol")
                    rc = ropep.tile([128, 1024], F16, tag="rope")
                    if tt == 0:
                        # first w chunk, then x/rope so matmuls start early
                        nc.sync.dma_start(w_sb[:, 0, :], wts[0])
                        nc.sync.dma_start(xcol[:], xts[tt])
                        nc.sync.dma_start(rc[:], rope[tt])
                        for dt in range(1, DT):
                            nc.sync.dma_start(w_sb[:, dt, :], wts[dt])
                        nc.sync.dma_start(id_sb[:], ident[:])
                        nc.sync.dma_start(ones_r[:], onesrow[:])
                        if with_bias:
                            nc.sync.dma_start(bias_sb[:], biasq[:])
                            nc.sync.dma_start(ones_1r[:], ones1r[:])
                    else:
                        nc.sync.dma_start(xcol[:], xts[tt])
                        nc.sync.dma_start(rc[:], rope[tt])
                    if tt == 2:
                        nc.sync.dma_start(masks_sb[:], masks[:])
                        nc.sync.dma_start(wp_sb[:], wpd[:])

                    psA = psAp.tile([128, 512], F32, tag="psA")
                    psB = psBp.tile([128, 260], F32, tag="psB")
                    for dt in range(DT):
                        nc.tensor.matmul(
                            psA[:], xcol[:, dt, :], w_sb[:, dt, 0:512],
                            start=(dt == 0),
                            stop=(dt == DT - 1 and not with_bias))
                        nc.tensor.matmul(
                            psB[:], xcol[:, dt, :], w_sb[:, dt, 512:772],
                            start=(dt == 0),
                            stop=(dt == DT - 1 and not with_bias))
                    if with_bias:
                        nc.tensor.matmul(
                            psA[:], ones_1r[:], bias_sb[:, 0:512],
                            start=False, stop=True)
                        nc.tensor.matmul(
                            psB[:], ones_1r[:], bias_sb[:, 512:772],
                            start=False, stop=True)

                    # transpose the PREVIOUS tile's qn while this tile's
                    # LN/RoPE chain runs -- keeps the PE stream unbroken.
                    if qn_prev is not None:
                        transpose_out(qn_prev)

                    nc.scalar.activation(
                        vv[:, 0:2, tt, :],
                        psB[:, 0:256].rearrange("p (h c) -> p h c", h=2),
                        AF.Copy)

                    st = lnstp.tile([128, 16], F32, tag="lnst")
                    # st cols: 0:4 mu, 4:8 sumsq, 8:12 rstd, 12:16 -mu*rstd
                    nc.vector.tensor_scalar(
                        st[:, 0:4], psB[:, 256:260], 1.0 / C, None,
                        op0=ALU.mult)
                    for i in range(4):
                        sq = sqp.tile([128, 128], F32, tag="sq")
                        nc.scalar.activation(
                            sq[:], psA[:, i * 128:(i + 1) * 128], AF.Square,
                            bias=zeros_c[:], accum_out=st[:, 4 + i:5 + i])
                    var = lnstp.tile([128, 4], F32, tag="var")
                    nc.vector.tensor_scalar(
                        var[:], st[:, 4:8], 1.0 / C, None, op0=ALU.mult)
                    mu2 = lnstp.tile([128, 4], F32, tag="mu2")
                    nc.vector.tensor_tensor(
                        mu2[:], st[:, 0:4], st[:, 0:4], op=ALU.mult)
                    nc.vector.tensor_tensor(
                        var[:], var[:], mu2[:], op=ALU.subtract)
                    nc.scalar.activation(var[:], var[:], AF.Sqrt,
                                         bias=eps_c[:])
                    nc.vector.reciprocal_approx_fast(st[:, 8:12], var[:])
                    nc.vector.tensor_tensor(
                        st[:, 12:16], st[:, 0:4], st[:, 8:12], op=ALU.mult)
                    nc.vector.tensor_scalar(
                        st[:, 12:16], st[:, 12:16], -1.0, None, op0=ALU.mult)
                    qn = qnp.tile([128, 512], F16, tag="qn")
                    for i in range(4):
                        nc.vector.tensor_scalar(
                            qn[:, i * 128:(i + 1) * 128],
                            psA[:, i * 128:(i + 1) * 128],
                            st[:, 8 + i:9 + i], st[:, 12 + i:13 + i],
                            op0=ALU.mult, op1=ALU.add)
                    # RoPE: swap-copies on DVE, combines on DVE
                    rot = rotp.tile([128, 512], F16, tag="rot")
                    qn3 = qn[:].rearrange("p (a b) -> p a b", b=2)
                    rot3 = rot[:].rearrange("p (a b) -> p a b", b=2)
                    nc.vector.tensor_scalar(
                        rot3[:, :, 0], qn3[:, :, 1], -1.0, None, op0=ALU.mult)
                    nc.vector.tensor_copy(rot3[:, :, 1], qn3[:, :, 0])
                    nc.vector.tensor_tensor(
                        qn[:], qn[:], rc[:, 0:512], op=ALU.mult)
                    nc.vector.tensor_tensor(
                        rot[:], rot[:], rc[:, 512:1024], op=ALU.mult)
                    nc.vector.tensor_tensor(qn[:], qn[:], rot[:], op=ALU.add)
                    qn_prev = (tt, qn)

                transpose_out(qn_prev)

            # =========== Phase B+C: attention + output projection ===========
            with (
                tc.tile_pool(name="psS", bufs=2, space="PSUM") as psSp,
                tc.tile_pool(name="psO", bufs=2, space="PSUM") as psOp,
                tc.tile_pool(name="psL", bufs=1, space="PSUM") as psLp,
                tc.tile_pool(name="psBC", bufs=1, space="PSUM") as psBCp,
                tc.tile_pool(name="psY", bufs=2, space="PSUM") as psYp,
                tc.tile_pool(name="aT", bufs=4) as aTp,
                tc.tile_pool(name="bsm", bufs=2) as bsmp,
                tc.tile_pool(name="ysb", bufs=2) as ysbp,
            ):
                # Deferred block tail: the bc matmul must wait on the DVE/ACT
                # recL chain; emitting it at the head of the PE queue stalls
                # the PE and drops it out of its max P-state.  Instead the
                # PE-touching tail ops are emitted in the middle of the NEXT
                # block's s-loop, where queued S-matmul work hides the wait.
                pending = []

                def emit_tail(h, tb, Ops, recLr):
                    bc = psBCp.tile([128, 512], F32, tag="psBC")
                    nc.tensor.matmul(bc[:], ones_r[:], recLr[:],
                                     start=True, stop=True)
                    bcs = bsmp.tile([128, 512], F32, tag="bcs")
                    nc.vector.tensor_copy(bcs[:], bc[:])
                    nc.vector.tensor_tensor(
                        ot[:, h, tb * 512:(tb + 1) * 512], Ops[:],
                        bcs[:], op=ALU.mult)

                def flush_tail():
                    while pending:
                        emit_tail(*pending.pop(0))

                def attn_block(h, tb):
                    S = 4 * (tb + 1)
                    qTs = qT[:, h, tb * 512:(tb + 1) * 512]

                    def lo_of(s):
                        # first nonzero column of s-tile s in this t-block
                        return (s - 4 * tb) * 128 if s >= 4 * tb else 0

                    Lps = psLp.tile([1, 512], F32, tag="psL")
                    Ops = psOp.tile([128, 512], F32, tag="psO")
                    st_ps = [None] * S

                    def emit_st(s):
                        stp = psSp.tile([128, 512], F32, tag="psS")
                        lo = lo_of(s)
                        nc.tensor.matmul(
                            stp[:, lo:512],
                            kT[:, h, s * 128:(s + 1) * 128], qTs[:, lo:512],
                            start=True, stop=True)
                        st_ps[s] = stp

                    emit_st(0)
                    if S > 1:
                        emit_st(1)
                    for s in range(S):
                        lo = lo_of(s)
                        a = aTp.tile([128, 512], F16, tag="aT")
                        nc.scalar.activation(
                            a[:, lo:512], st_ps[s][:, lo:512], AF.Exp,
                            bias=negb_c[:], scale=sc)
                        st_ps[s] = None
                        if s >= 4 * tb:
                            # only the [128,128] triangle block needs masking
                            nc.vector.tensor_tensor(
                                a[:, lo:lo + 128], a[:, lo:lo + 128],
                                masks_sb[:], op=ALU.mult)
                        if s + 2 < S:
                            emit_st(s + 2)
                        if s == 1:
                            flush_tail()
                        nc.tensor.matmul(
                            Lps[:, lo:512], ones_c[:], a[:, lo:512],
                            start=(s == 0), stop=(s == S - 1))
                        nc.tensor.matmul(
                            Ops[:, lo:512], vv[:, h, s, :], a[:, lo:512],
                            start=(s == 0), stop=(s == S - 1))
                    recL = bsmp.tile([1, 512], F32, tag="recL")
                    nc.vector.reciprocal_approx_fast(recL[:], Lps[:])
                    recLr = bsmp.tile([1, 512], F32R, tag="recLr")
                    nc.scalar.activation(recLr[:], recL[:], AF.Copy)
                    pending.append((h, tb, Ops, recLr))

                def proj_block(tb):
                    # y rows [tb*512, (tb+1)*512) need ot cols of this tb only
                    for ttt in range(4 * tb, 4 * tb + 4):
                        ysb = ysbp.tile([128, 4, 512], F16, tag="ysb")
                        for db in range(NB):
                            yps = psYp.tile([128, 512], F32, tag="psY")
                            for h in range(HPC):
                                nc.tensor.matmul(
                                    yps[:],
                                    ot[:, h, ttt * 128:(ttt + 1) * 128],
                                    wp_sb[:, h, db * 512:(db + 1) * 512],
                                    start=(h == 0), stop=(h == HPC - 1))
                            if db % 2 == 0:
                                nc.scalar.activation(
                                    ysb[:, db, :], yps[:], AF.Copy)
                            else:
                                nc.vector.tensor_copy(ysb[:, db, :], yps[:])
                        nc.sync.dma_start(
                            y[ttt * 128:(ttt + 1) * 128, :],
                            ysb[:].rearrange("p a b -> p (a b)"))

                for tb in range(NB):
                    attn_block(0, tb)
                    attn_block(1, tb)
                    if tb > 0:
                        proj_block(tb - 1)
                flush_tail()
                proj_block(NB - 1)

    nc.compile()
    return nc


def _host_prep(x, W_attn, b_attn, W_proj, q_ln_w, k_ln_w):
    f = np.float32
    h16 = np.float16

    # x pretiled: xts[tt, p, a, j] = x[tt*128+j, a*128+p]
    x4 = x.reshape(NT, 128, DT, 128)          # [tt, j, a, p]
    xts = np.ascontiguousarray(
        x4.transpose(0, 3, 2, 1).astype(h16))  # [tt, p, a, j]

    inv = (1.0 / (10000.0 ** (np.arange(0, C, 2, dtype=f) / C))).astype(f)
    freqs = np.arange(T, dtype=f)[:, None] * inv[None, :]
    sin = np.repeat(np.sin(freqs), 2, axis=1).astype(f)
    cos = np.repeat(np.cos(freqs), 2, axis=1).astype(f)
    part = np.arange(C) ^ 1
    cos_q = cos * q_ln_w[None, :]
    sin_q = sin * q_ln_w[None, part]
    cos_k = cos * k_ln_w[None, :]
    sin_k = sin * k_ln_w[None, part]
    ropecos = np.concatenate([cos_q, cos_q, cos_k, cos_k], axis=1)
    ropesin = np.concatenate([sin_q, sin_q, sin_k, sin_k], axis=1)
    ropetab = np.ascontiguousarray(
        np.concatenate([ropecos, ropesin], axis=1)
        .reshape(NT, 128, 1024).astype(h16))

    ss = np.arange(128)[:, None]
    ttm = np.arange(128)[None, :]
    masks = np.ascontiguousarray((ss <= ttm).astype(h16))

    with_bias = bool(np.any(b_attn != 0.0))

    shared = dict(xts=xts, rope=ropetab, masks=masks,
                  onesrow=np.ones((1, 128), f),
                  ident=np.eye(128, dtype=h16))
    if with_bias:
        shared["ones1r"] = np.ones((1, 128), h16)

    in_maps = []
    for c in range(NCORES):
        h0, h1 = HPC * c, HPC * c + 1
        rows = np.concatenate([
            np.arange(h0 * C, (h0 + 1) * C),
            np.arange(h1 * C, (h1 + 1) * C),
            D + np.arange(h0 * C, (h0 + 1) * C),
            D + np.arange(h1 * C, (h1 + 1) * C),
            2 * D + np.arange(h0 * C, (h0 + 1) * C),
            2 * D + np.arange(h1 * C, (h1 + 1) * C),
        ])
        wq = W_attn[rows].T                    # [D, 768]
        # 4 extra columns: per-head channel sums of the q/k blocks so the
        # LN mean comes out of the qkv matmul directly.
        wsum = wq[:, 0:512].reshape(D, 4, 128).sum(axis=2)   # [D, 4]
        wqa = np.concatenate([wq, wsum], axis=1)             # [D, 772]
        wts = np.ascontiguousarray(
            wqa.reshape(DT, 128, WQ).astype(h16))
        wpc = np.stack(
            [W_proj[:, h0 * C:(h0 + 1) * C].T,
             W_proj[:, h1 * C:(h1 + 1) * C].T], axis=0)  # [2, 128, D]
        wpd = np.ascontiguousarray(wpc.transpose(1, 0, 2).astype(h16))
        m = dict(shared)
        m["wts"] = wts
        m["wpd"] = wpd
        if with_bias:
            ba = b_attn[rows]
            bs = ba[0:512].reshape(4, 128).sum(axis=1)
            m["biasq"] = np.ascontiguousarray(
                np.concatenate([ba, bs])[None, :]).astype(h16)
        in_maps.append(m)
    return in_maps, with_bias


def kernel(x, W_attn, b_attn, W_proj, b_proj, q_ln_w, k_ln_w):
    global LAST_RESULT
    f = np.float32
    x = np.ascontiguousarray(np.asarray(x, f))
    W_attn = np.ascontiguousarray(np.asarray(W_attn, f))
    b_attn = np.ascontiguousarray(np.asarray(b_attn, f))
    W_proj = np.ascontiguousarray(np.asarray(W_proj, f))
    b_proj = np.ascontiguousarray(np.asarray(b_proj, f))
    q_ln_w = np.ascontiguousarray(np.asarray(q_ln_w, f))
    k_ln_w = np.ascontiguousarray(np.asarray(k_ln_w, f))

    in_maps, with_bias = _host_prep(x, W_attn, b_attn, W_proj, q_ln_w, k_ln_w)
    if with_bias not in _NC_CACHE:
        _NC_CACHE[with_bias] = _build_program(with_bias)
    nc = _NC_CACHE[with_bias]

    res = bass_utils.run_bass_kernel_spmd(
        nc, in_maps, core_ids=list(range(NCORES)),
        trace=bool(os.environ.get("BASS_TRACE")))
    LAST_RESULT = res

    y = np.zeros((T, D), np.float32)
    for rmap in res.results:
        y += rmap["y"].astype(np.float32)
    y += b_proj[None, :]
    return y
